# revision 30
# baseline (speedup 1.0000x reference)
"""GAT (5-layer, dense-adjacency) Trainium2 kernel, sharded across 8 NeuronCores.

Sharding: query-node rows split 512/core. Per layer each core computes its
own row-block of the augmented projection [Wh | ones | d] (the ones column
yields softmax denominators straight out of the attention matmul), AllGathers
it in bf16 (one collective per layer, overlapped with the score pipeline via
deep work buffering), then computes its row-block of attention: fused DVE op
(leaky(s+d)+addmask) + ACT exp + one bf16 matmul per (head, j-tile).
"""

import numpy as np

import concourse.bacc as bacc
import concourse.mybir as mybir
import concourse.tile as tile
from concourse.bass_utils import run_bass_kernel_spmd

import concourse.dve_ops as dve_ops
from concourse.dve_spec import Spec, Src0, Src1, C0, C1, maxx, lower
from concourse.dve_spec import _has_src1 as _spec_has_src1
from concourse.dve_uop import DveOpSpec

try:
    import ml_dtypes

    _BF16 = ml_dtypes.bfloat16
except ImportError:  # pragma: no cover
    _BF16 = np.float32

dt = mybir.dt
AF = mybir.ActivationFunctionType

# ---------------------------------------------------------------- constants
N = 4096
NCORE = 8
ROWS = N // NCORE  # 512 query rows per core
P = 128
JT = N // P  # 32 j-tiles
NEG = -30000.0  # additive mask for non-edges; exp(x-30000) == 0
ALPHA = 0.1
# (fin, fout, heads, concat, elu_after, group_size)
CFG = [
    (256, 128, 8, True, True, 4),
    (128, 64, 8, True, True, 4),
    (64, 32, 4, True, True, 2),
    (32, 16, 1, True, False, 1),
    (16, 8, 1, False, False, 1),
]

# ---------------------------------------------------------------- custom op
LEAKY_BIAS_ADDMASK = dve_ops.DveOp(
    "LEAKY_BIAS_ADDMASK",
    Spec(
        body=maxx(Src0 + C0, (Src0 + C0) * C1) + Src1,
        reference=lambda in0, in1, s0, s1, imm2: (
            np.maximum(in0 + s0, (in0 + s0) * s1) + in1
        ).astype(np.float32),
    ),
    subdim=False,
    uops_sha={},
)


def _register_custom_op(op):
    if op.name in dve_ops._SUB_OPCODE_FOR_NAME:
        return
    idx = dve_ops._CUSTOM_DVE_ROW_BASE + len(dve_ops.OPS)
    assert idx < 0x20
    dve_ops.OPS.append(op)
    dve_ops.CUSTOM_DVE_SPECS[op.name] = op.spec
    dve_ops._SUB_OPCODE_FOR_NAME[op.name] = idx
    shas = {}
    for ver in ("v3", "v4"):
        try:
            s = DveOpSpec(
                name=op.name,
                opcode=idx,
                uops=lower(op.spec, ver=ver),
                rd1_en=_spec_has_src1(op.spec),
            )
            shas[ver] = s.sha(ver)
        except Exception:
            pass
    object.__setattr__(op, "uops_sha", shas)


_register_custom_op(LEAKY_BIAS_ADDMASK)


def _groups(h, g):
    return [list(range(g0, min(g0 + g, h))) for g0 in range(0, h, g)]


# ---------------------------------------------------------------- builder
def build_kernel():
    import os as _os

    debug_taps = bool(_os.environ.get("DEBUG_TAPS"))
    nc = bacc.Bacc("TRN2", target_bir_lowering=False, debug=False)

    adjrows = nc.dram_tensor("adjrows", [ROWS, N], dt.int32, kind="ExternalInput")
    x0T_own = nc.dram_tensor("x0T_own", [256, ROWS], dt.float32, kind="ExternalInput")
    wext_dram = {}
    ws_dram = {}
    for li, (fin, fout, h, concat, _elu, _g) in enumerate(CFG, start=1):
        dh = fout // h if concat else fout
        cw2 = h * (dh + 1) + h  # per-head [values | 0(->1)] blocks, then d cols
        wext_dram[li] = nc.dram_tensor(
            f"wext{li}", [fin, cw2], dt.float32, kind="ExternalInput"
        )
        ws_dram[li] = nc.dram_tensor(f"ws{li}", [fin, h], dt.float32, kind="ExternalInput")

    pool_out = nc.dram_tensor("pool_part", [8, 1], dt.float32, kind="ExternalOutput")
    if debug_taps:
        dbg_x = {}
        for _li, (_f, _fo, _h, _c, _e, _g) in enumerate(CFG, start=1):
            dbg_x[_li] = nc.dram_tensor(
                f"dbg_x{_li}", [_fo, ROWS], dt.float32, kind="ExternalOutput"
            )

    ident_np = np.eye(P, dtype=_BF16)
    ident_dram = nc.inline_tensor(ident_np, name="ident128")

    with tile.TileContext(nc) as tc:
        with (
            tc.tile_pool(name="persist", bufs=1) as persist,
            tc.tile_pool(name="dram", bufs=1, space="DRAM") as drampool,
            tc.tile_pool(name="xTown", bufs=3) as xTown_pool,
            tc.tile_pool(name="layerbuf", bufs=2) as layerbuf,
            tc.tile_pool(name="ownp", bufs=2) as ownp,
            tc.tile_pool(name="srep", bufs=1) as srep_pool,
            tc.tile_pool(name="work", bufs=2) as work,
            tc.tile_pool(name="pjt", bufs=4) as pjt_pool,
            tc.tile_pool(name="pjts", bufs=8) as pjts_pool,
            tc.tile_pool(name="small", bufs=2) as small,
            tc.tile_pool(name="whps", bufs=1, space="PSUM") as whps,
            tc.tile_pool(name="sps", bufs=1, space="PSUM") as sps,
            tc.tile_pool(name="trps", bufs=1, space="PSUM") as trps,
            tc.tile_pool(name="attps", bufs=4, space="PSUM") as attps,
        ):
            # ---------------- persistent tiles
            maskT = persist.tile([P, JT, ROWS], dt.bfloat16, tag="maskT")
            ident_sb = persist.tile([P, P], dt.bfloat16, tag="ident")
            nc.sync.dma_start(ident_sb[:], ident_dram[:])
            ones_row = persist.tile([1, P], dt.float32, tag="ones_row")
            nc.vector.memset(ones_row[:], 1.0)
            ones_blk = persist.tile([P, 16], dt.float32, tag="ones_blk")
            nc.vector.memset(ones_blk[:], 1.0)

            wext_sb = {}
            ws_sb = {}
            for li, (fin, fout, h, concat, _elu, _g) in enumerate(CFG, start=1):
                dh = fout // h if concat else fout
                cw2 = h * (dh + 1) + h
                nft = (fin + P - 1) // P
                wext_sb[li] = []
                ws_sb[li] = []
                for ft in range(nft):
                    fr = min(P, fin - ft * P)
                    wt = persist.tile([fr, cw2], dt.float32, tag=f"wext{li}_{ft}")
                    nc.sync.dma_start(wt[:], wext_dram[li][ft * P : ft * P + fr, :])
                    wext_sb[li].append(wt)
                    st = persist.tile([fr, h], dt.float32, tag=f"ws{li}_{ft}")
                    nc.sync.dma_start(st[:], ws_dram[li][ft * P : ft * P + fr, :])
                    ws_sb[li].append(st)

            # ---------------- L1 own activations from input
            xTown_cur = []
            for ft in range(2):
                to = xTown_pool.tile([P, ROWS], dt.float32, tag="xTown")
                nc.sync.dma_start(to[:], x0T_own[ft * P : (ft + 1) * P, :])
                xTown_cur.append(to)

            def build_mask(chunks):
                # transpose adj rows -> additive maskT (bf16).
                CH = 1024
                for c0 in chunks:
                    for ib in range(ROWS // P):
                        stage_i = work.tile([P, CH], dt.int32, tag="stage_i")
                        # ACT's DGE queue: keeps this bulk stream from blocking
                        # the latency-critical gather/unpack DMAs on SP's queue
                        nc.scalar.dma_start(
                            stage_i[:], adjrows[ib * P : (ib + 1) * P, c0 : c0 + CH]
                        )
                        stage_b = work.tile([P, CH], dt.bfloat16, tag="stage_b")
                        nc.gpsimd.tensor_copy(stage_b[:], stage_i[:])
                        for k in range(CH // P):
                            jt = (c0 + k * P) // P
                            tps = trps.tile([P, P], dt.bfloat16, tag="tps")
                            nc.tensor.transpose(
                                tps[:], stage_b[:, k * P : (k + 1) * P], ident_sb[:]
                            )
                            # adj -> additive mask: adj*(-NEG) + NEG
                            nc.vector.tensor_scalar(
                                maskT[:, jt, ib * P : (ib + 1) * P],
                                tps[:],
                                -NEG,
                                NEG,
                                mybir.AluOpType.mult,
                                mybir.AluOpType.add,
                            )

            for li, (fin, fout, h, concat, elu, G) in enumerate(CFG, start=1):
                dh = fout // h if concat else fout
                dh1 = dh + 1
                cw2 = h * dh1 + h
                nft = (fin + P - 1) // P
                is_last = li == len(CFG)
                groups = _groups(h, G)

                # ---- own-block [Wh+ones (group-major) | all d cols].
                # The tiny d-only AllGather launches FIRST (scores gate on it);
                # vals AllGathers follow per group (matmuls gate on those).
                pw_all = ownp.tile([P, 4, cw2], dt.float32r, tag="own_sb")
                for k in range(4):
                    pw = whps.tile([P, cw2], dt.float32, tag="pw")
                    for ft in range(nft):
                        fr = min(P, fin - ft * P)
                        nc.tensor.matmul(
                            pw[:],
                            xTown_cur[ft][0:fr, k * P : (k + 1) * P],
                            wext_sb[li][ft][:],
                            start=(ft == 0),
                            stop=(ft == nft - 1),
                        )
                    nc.scalar.copy(pw_all[:, k, :], pw[:])
                goff = [gs[0] * dh1 for gs in groups]
                # d-only gather
                agd_in = drampool.tile([4 * P, h], dt.float32, tag=f"agdin{li}")
                agd_out = drampool.tile(
                    [NCORE, 4 * P, h],
                    dt.float32,
                    tag=f"agdout{li}",
                    addr_space="Shared",
                )
                nc.sync.dma_start(
                    agd_in.rearrange("(k p) c -> p k c", p=P),
                    pw_all[:, :, h * dh1 : cw2].bitcast(dt.float32),
                )
                nc.gpsimd.collective_compute(
                    "AllGather",
                    mybir.AluOpType.bypass,
                    replica_groups=[list(range(NCORE))],
                    ins=[agd_in.opt()],
                    outs=[agd_out.opt()],
                )
                # vals gathers, group A first
                ag_outs = []
                for gi, gs in enumerate(groups):
                    ng = len(gs)
                    cwg = ng * dh1
                    vals = pw_all[:, :, goff[gi] : goff[gi] + cwg].rearrange(
                        "p k (a b) -> p k a b", a=ng
                    )
                    nc.scalar.copy(
                        vals[:, :, :, dh : dh + 1],
                        ones_blk[:, 0 : 4 * ng].rearrange(
                            "p (k a b) -> p k a b", k=4, a=ng
                        ),
                    )
                    ag_in = drampool.tile(
                        [4 * P, cwg], dt.float32r, tag=f"agin{li}g{gi}"
                    )
                    ag_out = drampool.tile(
                        [NCORE, 4 * P, cwg],
                        dt.float32r,
                        tag=f"agout{li}g{gi}",
                        addr_space="Shared",
                    )
                    nc.sync.dma_start(
                        ag_in.rearrange("(k p) c -> p k c", p=P),
                        pw_all[:, :, goff[gi] : goff[gi] + cwg],
                    )
                    nc.gpsimd.collective_compute(
                        "AllGather",
                        mybir.AluOpType.bypass,
                        replica_groups=[list(range(NCORE))],
                        ins=[ag_in.opt()],
                        outs=[ag_out.opt()],
                    )
                    ag_outs.append(ag_out)

                # ---- unpack d (scores' gate) then sreps; mask interleaved at L1
                d_sb = layerbuf.tile([P, JT, h], dt.float32, tag="d_sb")
                nc.sync.dma_start(
                    d_sb[:], agd_out.rearrange("r (k p) c -> p (r k) c", p=P)
                )
                d_g = [
                    d_sb.rearrange("p j c -> p j c")[:, :, gs[0] : gs[0] + len(gs)]
                    for gs in groups
                ]

                mask_chunks = list(range(0, N, 1024))

                def emit_sreps(hhs):
                    for hh in hhs:
                        ps_row = sps.tile([1, ROWS], dt.float32, tag="ps_row")
                        for ft in range(nft):
                            fr = min(P, fin - ft * P)
                            nc.tensor.matmul(
                                ps_row[:],
                                ws_sb[li][ft][:, hh : hh + 1],
                                xTown_cur[ft][0:fr, :],
                                start=(ft == 0),
                                stop=(ft == nft - 1),
                            )
                        s_row = small.tile([1, ROWS], dt.float32, tag="vec1")
                        nc.scalar.copy(s_row[:], ps_row[:])
                        srt = srep_pool.tile(
                            [P, ROWS], dt.float32, tag=f"srep{hh}", name=f"sr{hh}"
                        )
                        nc.gpsimd.partition_broadcast(srt[:], s_row[:])
                        sreps[hh] = srt

                sreps = {}
                if li == 1:
                    build_mask(mask_chunks[0:1])
                    emit_sreps(range(h))
                    build_mask(mask_chunks[1:])
                else:
                    emit_sreps(range(h))

                # ---- unpack vals per group (one DMA each)
                whrow_g = []
                for gi, gs in enumerate(groups):
                    ng = len(gs)
                    cwg = ng * dh1
                    wr = layerbuf.tile(
                        [P, JT, cwg], dt.float32r, tag=f"whrow{gi}", name=f"wr{gi}"
                    )
                    nc.sync.dma_start(
                        wr[:], ag_outs[gi].rearrange("r (k p) c -> p (r k) c", p=P)
                    )
                    whrow_g.append(wr)

                # ---- attention per head group
                xnext = xTown_pool.tile([fout, ROWS], dt.float32, tag="xTown")
                n_groups = len(groups)
                for gi, gs in enumerate(groups):
                    ng = len(gs)
                    last_group = gi == n_groups - 1
                    att_acc = {}
                    for hh in gs:
                        att_acc[hh] = attps.tile(
                            [dh1, ROWS], dt.float32, tag="att", name=f"att{hh}"
                        )
                    for jt in range(JT):
                        l_jt = work.tile([P, ng * ROWS], dt.float32, tag="l_jt")
                        for k, hh in enumerate(gs):
                            nc.vector._custom_dve(
                                LEAKY_BIAS_ADDMASK,
                                out=l_jt[:, k * ROWS : (k + 1) * ROWS],
                                in0=sreps[hh][:],
                                in1=maskT[:, jt, :],
                                s0=d_g[gi][:, jt, k : k + 1],
                                s1=ALPHA,
                            )
                        pool_p = pjts_pool if h == 1 else pjt_pool
                        p_jt = pool_p.tile([P, ng * ROWS], dt.float32r, tag="p_jt")
                        nc.scalar.activation(p_jt[:], l_jt[:], AF.Exp)
                        for k, hh in enumerate(gs):
                            nc.tensor.matmul(
                                att_acc[hh][:],
                                whrow_g[gi][:, jt, k * dh1 : (k + 1) * dh1],
                                p_jt[:, k * ROWS : (k + 1) * ROWS],
                                start=(jt == 0),
                                stop=(jt == JT - 1),
                            )
                    # epilogue per head; last group splits across engines to
                    # shorten the inter-layer critical path
                    for k, hh in enumerate(gs):
                        dve_path = last_group and (k % 2 == 0)
                        o_sb = small.tile([dh1, ROWS], dt.float32, tag="o_sb")
                        nc.scalar.copy(o_sb[:], att_acc[hh][:])
                        # compute engines can't read at partition offset dh
                        # (must be 0/32/64/96); DMA the denominator row down
                        r_in = small.tile([1, ROWS], dt.float32, tag="vec1i")
                        nc.sync.dma_start(r_in[:], o_sb[dh : dh + 1, :])
                        r_sb = small.tile([1, ROWS], dt.float32, tag="vec1")
                        nc.vector.reciprocal(r_sb[:], r_in[:])
                        ohead = small.tile([dh, ROWS], dt.float32, tag="ohead")
                        if dve_path:
                            rps = trps.tile([dh, ROWS], dt.float32, tag="rps")
                            nc.tensor.matmul(
                                rps[:], ones_row[0:1, 0:dh], r_sb[:],
                                start=True, stop=True,
                            )
                            nc.vector.tensor_mul(ohead[:], o_sb[0:dh, :], rps[:])
                        else:
                            rrep = small.tile([dh, ROWS], dt.float32, tag="rrep")
                            nc.gpsimd.partition_broadcast(rrep[:], r_sb[:])
                            nc.gpsimd.tensor_mul(ohead[:], o_sb[0:dh, :], rrep[:])
                        if elu:
                            # elu(x) = max(x,0) - 1 + exp(min(x,0))
                            mmin = small.tile([dh, ROWS], dt.float32, tag="tmp1")
                            emin = small.tile([dh, ROWS], dt.float32, tag="tmp2")
                            rmax = small.tile([dh, ROWS], dt.float32, tag="tmp3")
                            if dve_path:
                                nc.vector.tensor_scalar(
                                    mmin[:], ohead[:], 0.0, None, mybir.AluOpType.min
                                )
                                nc.scalar.activation(emin[:], mmin[:], AF.Exp)
                                nc.vector.tensor_scalar(
                                    rmax[:], ohead[:], 0.0, -1.0,
                                    mybir.AluOpType.max, mybir.AluOpType.add,
                                )
                                nc.vector.tensor_add(ohead[:], rmax[:], emin[:])
                            else:
                                nc.gpsimd.tensor_scalar(
                                    mmin[:], ohead[:], 0.0, None, mybir.AluOpType.min
                                )
                                nc.scalar.activation(emin[:], mmin[:], AF.Exp)
                                nc.gpsimd.tensor_scalar(
                                    rmax[:], ohead[:], 0.0, -1.0,
                                    mybir.AluOpType.max, mybir.AluOpType.add,
                                )
                                nc.gpsimd.tensor_add(ohead[:], rmax[:], emin[:])
                        nc.sync.dma_start(xnext[hh * dh : (hh + 1) * dh, :], ohead[:])

                if debug_taps:
                    nc.sync.dma_start(dbg_x[li][:], xnext[:])
                if is_last:
                    psum_final = small.tile([fout, 1], dt.float32, tag="vec1f")
                    nc.vector.reduce_sum(
                        psum_final[:], xnext[:], axis=mybir.AxisListType.X
                    )
                    nc.sync.dma_start(pool_out[:], psum_final[:])
                else:
                    xTown_cur = [xnext]

    nc.finalize()
    return nc


_NC_CACHE = None
_last_in_maps = None


def kernel(**inputs):
    global _NC_CACHE
    node_features = np.asarray(inputs["node_features"], dtype=np.float32)
    adj = np.ascontiguousarray(np.asarray(inputs["adj_mat"], dtype=np.int32))
    fc_w = np.asarray(inputs["fc_w"], dtype=np.float32)
    fc_b = np.asarray(inputs["fc_b"], dtype=np.float32)

    x0T = node_features.T  # [256, N]

    wext = {}
    ws = {}
    for li, (fin, fout, h, concat, _elu, _g) in enumerate(CFG, start=1):
        dh = fout // h if concat else fout
        W = np.asarray(inputs[f"W{li}"], dtype=np.float32)  # [h, fin, dh]
        a_src = np.asarray(inputs[f"a_src{li}"], dtype=np.float32)  # [h, dh]
        a_dst = np.asarray(inputs[f"a_dst{li}"], dtype=np.float32)
        wd = np.einsum("hfd,hd->fh", W, a_dst).astype(np.float32)  # [fin, h]
        # augmented: group-major per-head [W_h | zero(->ones)] blocks,
        # then ALL d columns at the end (d ships in its own tiny gather)
        waug = np.zeros((fin, h * (dh + 1) + h), dtype=np.float32)
        for hh in range(h):
            waug[:, hh * (dh + 1) : hh * (dh + 1) + dh] = W[hh].reshape(fin, dh)
        waug[:, h * (dh + 1) :] = wd
        wext[li] = np.ascontiguousarray(waug)
        ws[li] = np.ascontiguousarray(
            np.einsum("hfd,hd->fh", W, a_src).astype(np.float32)
        )

    in_maps = []
    for c in range(NCORE):
        m = {
            "adjrows": np.ascontiguousarray(adj[c * ROWS : (c + 1) * ROWS, :]),
            "x0T_own": np.ascontiguousarray(x0T[:, c * ROWS : (c + 1) * ROWS]),
        }
        for li in range(1, 6):
            m[f"wext{li}"] = wext[li]
            m[f"ws{li}"] = ws[li]
        in_maps.append(m)

    if _NC_CACHE is None:
        _NC_CACHE = build_kernel()
    nc = _NC_CACHE
    global _last_in_maps
    _last_in_maps = in_maps

    res = run_bass_kernel_spmd(nc, in_maps, list(range(NCORE)))
    total = np.zeros((8,), dtype=np.float32)
    for c in range(NCORE):
        total += res.results[c]["pool_part"][:, 0]
    pooled = total / np.float32(N)
    out = pooled @ fc_w + fc_b
    return out.astype(np.float32)
